# revision 19
# baseline (speedup 1.0000x reference)
"""Trainium2 Bass kernel for 0.7*BCEWithLogits + 0.3*MultiLabelMarginLoss.

Math (per row of N = B*T rows, V = 128 classes; output = mean over rows):
  bce_row = mean_n[ softplus(x_n) - x_n*t_n ]
            softplus(x) = relu(x) + log1p(exp(-|x|));  sum relu = (sum x + sum |x|)/2
  mlm_row = (1/V) sum_{p in pos} sum_{n in neg} relu(1 - x_p + x_n)

Only global sums matter (scalar output), so reductions accumulate into
per-block/per-group columns or PSUM and combine once per core.

Positive logits (<= ~11 per row here) are extracted per 128-row block with
vector.max (top-8, sorted) + match_replace + vector.max into a raw table
t' = x_pos + 512 (pads = 0). The V^2 pairwise hinge collapses to S slots
per row, one fused custom DVE instruction per block:
    z[p,k,n] = select(t'[p,k] > 256, relu(u[p,n] - t'[p,k] + 513), 0)
    accum_out[p] = sum z        (u = x with positives pushed to -512)
A second custom op folds sum(x*t) per 4-block group into one instruction.

Everything else is batched per 4-block group to amortize fixed costs:
one 512 KiB DMA; one gpsimd tensor_tensor each for x+512, *(pos), u over
[128, 512] strided views; one Abs/Exp/Ln chain on ACT over [128, 512] with
group accum_out (single pinned table set); one PE column-sum matmul for x.

Sharding: host sorts rows by positive count, deals them round-robin to the
8 cores (identical npos profile per core), interleaves x|targets, and lays
the core's 16 blocks side-by-side as a [128, 16*256] array so each group is
one contiguous DMA. Block b needs S_b hinge slots; the schedule derives from
the npos histogram, one cached NEFF per distinct schedule. All arithmetic is
on device; the host only permutes/shards and sums the 8 core partials.
"""

import sys

sys.path.insert(0, "/opt/trn_rl_repo")

import numpy as np

import concourse.bacc as bacc
import concourse.tile as tile
from concourse import mybir
from concourse.bass_utils import run_bass_kernel_spmd

F32 = mybir.dt.float32
ALU = mybir.AluOpType
ACTF = mybir.ActivationFunctionType
AXL = mybir.AxisListType

B, T, V = 16, 1024, 128
ROWS = B * T
N_CORES = 8
RPC = ROWS // N_CORES             # 2048 rows per core
P = 128                           # rows per block
NBLK = RPC // P                   # 16 blocks
GRP = 4                           # blocks per group
NGRP = NBLK // GRP
CB = 2 * V                        # columns per block in the packed layout
CG = GRP * CB                     # columns per group

BIG = 512.0
BCE_W = 0.7
MLM_W = 0.3


def _register_ops():
    from concourse import dve_ops as dops
    from concourse.dve_spec import (
        Spec, Src0, Src1, AluOp, relu, select, Zero, One, C0, C1,
    )

    if hasattr(dops, "ANT_KERNEL_OPS"):
        return dops.ANT_KERNEL_OPS

    def _zref(in0, in1, c0, c1, c2):
        i0 = in0.astype(np.float32).reshape(in0.shape[0], -1)
        t = in1.astype(np.float32).reshape(in0.shape[0], -1)
        b = np.where(t > c0, np.maximum(i0 - t + c1, 0.0), 0.0)
        return b, b.sum(-1, keepdims=True)

    z_spec = Spec(
        body=select(Src1 > C0, relu(Src0 - Src1 + C1), Zero),
        accum=AluOp.ADD, reference=_zref,
    )

    def _xtref(in0, in1, c0, c1, c2):
        t = in0.astype(np.float32).reshape(in0.shape[0], -1)
        b = np.where(t > c0, t - c1, 0.0)
        return b, b.sum(-1, keepdims=True)

    xt_spec = Spec(
        body=select(Src0 > C0, Src0 - C1, Zero),
        accum=AluOp.ADD, reference=_xtref,
    )

    def _pxref(in0, in1, c0, c1, c2):
        i0 = in0.astype(np.float32).reshape(in0.shape[0], -1)
        i1 = in1.astype(np.float32).reshape(in0.shape[0], -1)
        return (i0 + c0) * i1

    px_spec = Spec(body=(Src0 + C0) * Src1, reference=_pxref)

    def _uoref(in0, in1, c0, c1, c2):
        i0 = in0.astype(np.float32).reshape(in0.shape[0], -1)
        i1 = in1.astype(np.float32).reshape(in0.shape[0], -1)
        return i0 * (1.0 - i1) - c0 * i1

    uo_spec = Spec(body=Src0 * (One - Src1) - C0 * Src1, reference=_uoref)

    ops = {}
    for name, spec in (
        ("Z_HINGE2_ANT", z_spec),
        ("XT_SUM_ANT", xt_spec),
        ("PX_MASK_ANT", px_spec),
        ("U_MASK_ANT", uo_spec),
    ):
        opc = max(dops._SUB_OPCODE_FOR_NAME.values()) + 1
        shas = {}
        for ver in ("v3", "v4"):
            r = dops.DveOpSpec(
                name=name, opcode=opc,
                uops=dops.lower(spec, ver=ver), rd1_en=dops.has_src1(spec),
            )
            shas[ver] = r.sha(ver)
        op = dops.DveOp(name, spec, subdim=False, uops_sha=shas)
        dops.OPS.append(op)
        dops.CUSTOM_DVE_SPECS[name] = spec
        dops._SUB_OPCODE_FOR_NAME[name] = opc
        ops[name] = op
    dops.ANT_KERNEL_OPS = ops
    return ops


_OPS = _register_ops()
Z_HINGE = _OPS["Z_HINGE2_ANT"]
XT_SUM = _OPS["XT_SUM_ANT"]
PX_MASK = _OPS["PX_MASK_ANT"]
U_MASK = _OPS["U_MASK_ANT"]


def _act_set_id(nc):
    from concourse.hw_specs import get_activation_tables

    return list(get_activation_tables(nc.m.arch)).index("natural_log_exp_and_others")


def build_nc(schedule):
    """schedule: tuple of per-block hinge-slot counts (>= 1)."""
    nc = bacc.Bacc("TRN2", target_bir_lowering=False, debug=False)
    xp_dram = nc.dram_tensor("xp", [P, NBLK * CB], F32, kind="ExternalInput")
    out_dram = nc.dram_tensor("out", [1, 1], F32, kind="ExternalOutput")
    xp_ap = xp_dram.ap()

    with tile.TileContext(nc) as tc:
        with (
            tc.tile_pool(name="const", bufs=1) as cpool,
            tc.tile_pool(name="inp", bufs=3) as ipool,
            tc.tile_pool(name="work", bufs=3) as wpool,
            tc.tile_pool(name="zp", bufs=3) as zpool,
            tc.tile_pool(name="tt", bufs=2) as tpool,
            tc.tile_pool(name="accs", bufs=1) as apool,
            tc.tile_pool(name="ps", bufs=1, space="PSUM") as pspool,
        ):
            nc.scalar.add_instruction(
                mybir.InstLoadActFuncSet(
                    name=nc.get_next_instruction_name(), ins=[], outs=[],
                    act_func_set_id=_act_set_id(nc),
                )
            )
            ones = cpool.tile([P, 1], F32, tag="ones")
            nc.vector.memset(ones[:], 1.0)
            hcols = apool.tile([P, NBLK], F32, tag="hcols")
            xtg = apool.tile([P, NGRP], F32, tag="xtg")
            acols = apool.tile([P, 6], F32, tag="acols")
            lcols = apool.tile([P, 6], F32, tag="lcols")
            cs_x = pspool.tile([1, 4 * V], F32, tag="cs_x")
            cs_x0 = pspool.tile([1, V], F32, tag="cs_x0")

            # supergroups for DMA/PX/UO/ACT batching: blocks 0-3 per-block
            # (pipeline fill), 4-7 as one unit, 8-15 as one unit
            SGS = [list(range(0, 4)), list(range(4, 8)), list(range(8, 16))]
            acol_i = 0
            first_px = {}
            first_u = {}
            for sgi, blocks in enumerate(SGS):
                nb = len(blocks)
                b0 = blocks[0]
                if sgi == 0:
                    for j in blocks:
                        xb = ipool.tile([P, CB], F32, tag="xb")
                        nc.sync.dma_start(
                            xb[:], xp_ap[:, j * CB : (j + 1) * CB]
                        )
                        x = xb[:, 0:V]
                        pos = xb[:, V:CB]
                        pxb = wpool.tile([P, V], F32, tag="pxb")
                        nc.vector._custom_dve(
                            PX_MASK, out=pxb[:], in0=x, in1=pos, s0=BIG
                        )
                        ub = wpool.tile([P, V], F32, tag="ub")
                        nc.vector._custom_dve(
                            U_MASK, out=ub[:], in0=x, in1=pos, s0=BIG
                        )
                        first_px[j] = pxb[:]
                        first_u[j] = ub[:]
                        nc.tensor.matmul(
                            cs_x0[:], ones[:], x,
                            start=(j == 0), stop=(j == 3),
                        )
                        a = wpool.tile([P, V], F32, tag="a0")
                        nc.scalar.activation(
                            a[:], x, ACTF.Abs, bias=0.0, scale=1.0,
                            accum_out=acols[:, acol_i : acol_i + 1],
                        )
                        e = wpool.tile([P, V], F32, tag="e0")
                        nc.scalar.activation(
                            e[:], a[:], ACTF.Exp, bias=0.0, scale=-1.0
                        )
                        lns = wpool.tile([P, V], F32, tag="l0")
                        nc.scalar.activation(
                            lns[:], e[:], ACTF.Ln, bias=1.0, scale=1.0,
                            accum_out=lcols[:, acol_i : acol_i + 1],
                        )
                        acol_i += 1
                    continue

                xg = ipool.tile([P, nb * CB], F32, tag=f"xg{sgi}")
                nc.sync.dma_start(
                    xg[:], xp_ap[:, b0 * CB : (b0 + nb) * CB]
                )
                xgv = xg[:].rearrange("p (j c) -> p j c", j=nb)
                x_all = xgv[:, :, 0:V]
                pos_all = xgv[:, :, V:CB]

                pxf = wpool.tile([P, nb * V], F32, tag=f"pxf{sgi}")
                pxv = pxf[:].rearrange("p (j c) -> p j c", j=nb)
                nc.vector._custom_dve(
                    PX_MASK, out=pxv, in0=x_all, in1=pos_all, s0=BIG
                )
                uf = wpool.tile([P, nb * V], F32, tag=f"uf{sgi}")
                ufv = uf[:].rearrange("p (j c) -> p j c", j=nb)
                nc.vector._custom_dve(
                    U_MASK, out=ufv, in0=x_all, in1=pos_all, s0=BIG
                )
                for idx, j in enumerate(blocks):
                    first_px[j] = pxf[:, idx * V : (idx + 1) * V]
                    first_u[j] = uf[:, idx * V : (idx + 1) * V]

                # PE column sums (N <= 512 per matmul for fp32)
                for c0 in range(0, nb * V, 4 * V):
                    cw = min(4 * V, nb * V - c0)
                    nc.tensor.matmul(
                        cs_x[:, 0:cw], ones[:],
                        pxv if False else x_all[:, c0 // V : (c0 + cw) // V, :],
                        start=(sgi == 1 and c0 == 0),
                        stop=(sgi == len(SGS) - 1 and c0 + cw == nb * V),
                    )

                af = wpool.tile([P, nb * V], F32, tag=f"af{sgi}")
                afv = af[:].rearrange("p (j c) -> p j c", j=nb)
                nc.scalar.activation(
                    afv, x_all, ACTF.Abs, bias=0.0, scale=1.0,
                    accum_out=acols[:, acol_i : acol_i + 1],
                )
                ef = wpool.tile([P, nb * V], F32, tag=f"ef{sgi}")
                nc.scalar.activation(ef[:], af[:], ACTF.Exp, bias=0.0, scale=-1.0)
                lf = wpool.tile([P, nb * V], F32, tag=f"lf{sgi}")
                nc.scalar.activation(
                    lf[:], ef[:], ACTF.Ln, bias=1.0, scale=1.0,
                    accum_out=lcols[:, acol_i : acol_i + 1],
                )
                acol_i += 1

            for g in range(NGRP):
                tfat = tpool.tile([P, GRP * 16], F32, tag="tfat")
                nc.gpsimd.memset(tfat[:], 0.0)

                # extraction per block
                for j in range(GRP):
                    blk = g * GRP + j
                    S = schedule[blk]
                    c0 = j * 16
                    pxb = first_px[blk]
                    rounds = (S + 7) // 8
                    nc.vector.max(tfat[:, c0 : c0 + 8], pxb)
                    src = pxb
                    for r in range(1, rounds):
                        mr = wpool.tile([P, V], F32, tag="mr")
                        nc.vector.match_replace(
                            mr[:], tfat[:, c0 + 8 * (r - 1) : c0 + 8 * r], src, 0.0
                        )
                        nc.vector.max(tfat[:, c0 + 8 * r : c0 + 8 * (r + 1)], mr[:])
                        src = mr[:]

                # sum of positive logits for the group, one op
                xt_scr = tpool.tile([P, GRP * 16], F32, tag="xt_scr")
                nc.vector._custom_dve(
                    XT_SUM, out=xt_scr[:], in0=tfat[:],
                    s0=BIG / 2, s1=BIG,
                    accum_out=xtg[:, g : g + 1],
                )

                # fused hinge per block
                for j in range(GRP):
                    blk = g * GRP + j
                    S = schedule[blk]
                    c0 = j * 16
                    zr = zpool.tile([P, S * V], F32, tag="zr")
                    zv = zr[:].rearrange("p (s n) -> p s n", s=S)
                    u_b = first_u[blk].unsqueeze(1).broadcast_to([P, S, V])
                    t_b = tfat[:, c0 : c0 + S].unsqueeze(2).broadcast_to([P, S, V])
                    nc.vector._custom_dve(
                        Z_HINGE, out=zv, in0=u_b, in1=t_b,
                        s0=BIG / 2, s1=BIG + 1.0,
                        accum_out=hcols[:, blk : blk + 1],
                    )

            # ---- end-of-core combine ----
            h1 = apool.tile([P, 1], F32, tag="h1")
            nc.vector.tensor_reduce(h1[:], hcols[:], AXL.X, ALU.add)
            xt1 = apool.tile([P, 1], F32, tag="xt1")
            nc.vector.tensor_reduce(xt1[:], xtg[:], AXL.X, ALU.add)
            a1 = apool.tile([P, 1], F32, tag="a1")
            nc.vector.tensor_reduce(a1[:], acols[:], AXL.X, ALU.add)
            l1 = apool.tile([P, 1], F32, tag="l1")
            nc.vector.tensor_reduce(l1[:], lcols[:], AXL.X, ALU.add)

            # w = 0.5*a1 + l1 - xt1 + (0.3/0.7)*h1  (per partition)
            w1 = apool.tile([P, 1], F32, tag="w1")
            nc.vector.scalar_tensor_tensor(
                w1[:], a1[:], 0.5, l1[:], ALU.mult, ALU.add
            )
            w2 = apool.tile([P, 1], F32, tag="w2")
            nc.vector.tensor_tensor(w2[:], w1[:], xt1[:], ALU.subtract)
            w3 = apool.tile([P, 1], F32, tag="w3")
            nc.vector.scalar_tensor_tensor(
                w3[:], h1[:], MLM_W / BCE_W, w2[:], ALU.mult, ALU.add
            )
            wps = pspool.tile([1, 1], F32, tag="wps")
            nc.tensor.matmul(wps[:], ones[:], w3[:], start=True, stop=True)
            wsb = apool.tile([1, 1], F32, tag="wsb")
            nc.scalar.copy(wsb[:], wps[:])

            csb = apool.tile([1, 4 * V], F32, tag="csb")
            nc.scalar.copy(csb[:], cs_x[:])
            sxa = apool.tile([1, 1], F32, tag="sxa")
            nc.vector.tensor_reduce(sxa[:], csb[:], AXL.X, ALU.add)
            csb0 = apool.tile([1, V], F32, tag="csb0")
            nc.scalar.copy(csb0[:], cs_x0[:])
            sxb = apool.tile([1, 1], F32, tag="sxb")
            nc.vector.tensor_reduce(sxb[:], csb0[:], AXL.X, ALU.add)
            sx = apool.tile([1, 1], F32, tag="sx")
            nc.vector.tensor_tensor(sx[:], sxa[:], sxb[:], ALU.add)
            t2 = apool.tile([1, 1], F32, tag="t2")
            nc.vector.scalar_tensor_tensor(
                t2[:], sx[:], 0.5, wsb[:], ALU.mult, ALU.add
            )
            o2 = apool.tile([1, 1], F32, tag="o2")
            nc.vector.tensor_scalar(o2[:], t2[:], BCE_W / V, None, ALU.mult)
            nc.sync.dma_start(out_dram.ap()[:, :], o2[:])

    nc.compile()
    return nc


_NC_CACHE = {}


def _get_nc(schedule):
    if schedule not in _NC_CACHE:
        _NC_CACHE[schedule] = build_nc(schedule)
    return _NC_CACHE[schedule]


def _shard(x, t):
    """npos-sorted round-robin shard, x|pos interleave, block-major packing.
    Returns (schedule, [per-core [P, NBLK*CB] arrays])."""
    npos = (t > 0.5).sum(axis=1)
    order = np.argsort(npos, kind="stable")
    npos_sorted = npos[order]
    schedule = tuple(
        max(1, int(npos_sorted[(b + 1) * (N_CORES * P) - 1])) for b in range(NBLK)
    )
    xp = np.concatenate([x, t], axis=1)[order]   # [ROWS, 256]
    shards = []
    for c in range(N_CORES):
        s = xp[c::N_CORES]                        # [RPC, 256] npos-sorted
        s = s.reshape(NBLK, P, CB).transpose(1, 0, 2).reshape(P, NBLK * CB)
        shards.append(np.ascontiguousarray(s))
    return schedule, shards


def kernel(logits: np.ndarray, targets: np.ndarray) -> np.ndarray:
    x = np.asarray(logits, dtype=np.float32).reshape(ROWS, V)
    t = np.asarray(targets, dtype=np.float32).reshape(ROWS, V)
    schedule, shards = _shard(x, t)
    nc = _get_nc(schedule)
    in_maps = [{"xp": shards[c]} for c in range(N_CORES)]
    res = run_bass_kernel_spmd(nc, in_maps, list(range(N_CORES)))
    total = sum(float(res.results[c]["out"][0, 0]) for c in range(N_CORES))
    return np.float32(total / ROWS)
